# revision 8
# baseline (speedup 1.0000x reference)
"""Trainium2 Bass kernel for nn_Attention_interaction (dense_transformer).

Math (per batch b, head h):
    q = l2norm(x);  S = (q @ q^T) / SCALE / attn_gamma;  P = softmax(S, -1)
    o = P @ y;  o2 = o @ W^T + bias;  out = w0*y + w1*o2
with w_i = exp(sum_gamma_i) / (exp(sum_gamma0) + exp(sum_gamma1)).

Sharding: batch dim B=8 across the 8 cores (1 batch x 8 heads per core).

v2 design notes:
  - S matmuls run in fp8e4 DoubleRow mode (contraction k=(partition,ktile),
    d = 2p+ktile). q is scaled+cast to fp8 token-major, 2-byte DMA-xbar
    transposes give a pair-interleaved [32,*] layout, and a GpSimd
    deinterleave pass produces the walrus-required [p, ktile, token-run]
    block layout. 4 heads' operands live at PE quadrant rows 0/32/64/96.
  - exp splits between ACT (native Exp) and DVE (Schraudolph on bf16 bit
    patterns: bits16 = round(s*128/ln2 + (127*128 - sigma)), ~3.3% max rel
    err, way inside the 2e-2 gate), balanced by a greedy time counter.
  - Heads run SEQUENTIALLY (not in pairs): PSUM = 3 S-chunk slots (6 banks)
    + one [128,1024] O tile (2 banks). More slots give the PE runway to
    stay continuously busy (p-state ramp to 2.4 GHz needs 3us of
    uninterrupted execution).
  - Softmax denominators ride in the O matmul's 65th row (ones column in
    ya); proj writes pj into the freed jc-half of the same O PSUM tile.
  - Epilogue: o2 = pj * rinv via one DVE tensor_mul with a stride-0
    broadcast AP; fin = o2 + w0*y on GpSimd; store.
  - q-norm squares/scales: group 0 on DVE (warmup, DVE idle), group 1 on
    GpSimd (steady state); Newton inverse-sqrt (1 iter) on DVE.
"""

import math

import numpy as np
import ml_dtypes

import concourse.bass as bass
import concourse.bacc as bacc
import concourse.tile as tile
from concourse import mybir
from concourse.bass_utils import run_bass_kernel_spmd
from concourse._compat import get_trn_type

B, H, N, D = 8, 8, 1024, 64
SCALE = (512 // 8) ** (-0.5)  # 0.125
EPS = 1e-6
NCORES = 8
NB = N // 128  # 8 row blocks of 128
NW = N * NB  # 8192 flattened S columns per head
CHUNK = 1024
F32 = mybir.dt.float32
BF16 = mybir.dt.bfloat16
FP8 = mybir.dt.float8e4
I16 = mybir.dt.int16
U8 = mybir.dt.uint8
I32 = mybir.dt.int32
AX = mybir.AxisListType
OP = mybir.AluOpType
ACT = mybir.ActivationFunctionType
PM = mybir.MatmulPerfMode
MAGIC = 0x5F3759DF

# Schraudolph exp on fp8e4 bit patterns: bits = round(s*A_SCH + B_SCH)
A_SCH = 8.0 / math.log(2.0)
SIGMA = 0.375
B_SCH = 7.0 * 8.0 - SIGMA

# greedy exp-engine balancing: estimated op costs in us
ACT_CHUNK = 1.223
DVE_CHUNK = 1.317
ACT_EVAC = 0.80
DVE_EVAC = 0.69
DVE_O2 = 0.78
DVE_RECIP = 0.30

LAST_RESULTS = None  # BassKernelResults of the most recent run (for test.py)


def _emit(ctx, tc, sqrt_c2: float):
    nc = tc.nc
    xq = nc.dram_tensor("xq", [H, N, D], BF16, kind="ExternalInput")
    ya = nc.dram_tensor("ya", [H, N, D + 1], FP8, kind="ExternalInput")
    yb = nc.dram_tensor("yb", [H, N, D], F32, kind="ExternalInput")
    wt = nc.dram_tensor("wt", [D + 1, D], BF16, kind="ExternalInput")
    out = nc.dram_tensor("out", [H, N, D], F32, kind="ExternalOutput")
    rscr = nc.dram_tensor("rscr", [H, N], BF16)  # denominator bounce

    singles = ctx.enter_context(tc.tile_pool(name="singles", bufs=1))
    io = ctx.enter_context(tc.tile_pool(name="io", bufs=2))
    st = ctx.enter_context(tc.tile_pool(name="st", bufs=2))
    qpool = ctx.enter_context(tc.tile_pool(name="qpool", bufs=1))
    epool = ctx.enter_context(tc.tile_pool(name="epool", bufs=2))
    wpool = ctx.enter_context(tc.tile_pool(name="wpool", bufs=2))
    ps_s = ctx.enter_context(tc.tile_pool(name="ps_s", bufs=3, space="PSUM"))
    ps_o = ctx.enter_context(tc.tile_pool(name="ps_o", bufs=1, space="PSUM"))

    qT4 = [None, None]  # per group: [128, 2048] fp8 block layout (as bf16 tile)
    eng_t = {"act": 0.0, "dve": 0.0}  # greedy engine-time counters

    def qprep(g):
        """q-prep for heads 4g..4g+3. Squares+scales on DVE for group 0
        (warmup) and on GpSimd for group 1 (steady state)."""
        eng = nc.vector
        x4 = io.tile([128, 4, NB, D], BF16, tag=f"x4_{g}", name=f"x4_{g}")
        nc.sync.dma_start(
            out=x4,
            in_=xq[4 * g : 4 * g + 4].rearrange("h (b p) d -> p h b d", p=128),
        )
        ss = st.tile([128, 4, NB], F32, tag=f"ss{g}", name=f"ss{g}")
        for hi in range(4):
            sq = st.tile([128, NB, D], BF16, tag=f"sq{g}", name=f"sq{g}")
            eng.tensor_mul(sq, x4[:, hi], x4[:, hi])
            nc.vector.tensor_reduce(ss[:, hi], sq, axis=AX.X, op=OP.add)

        # rs = sqrt_c2 / sqrt(ss + eps): fast inverse sqrt + 1 Newton (DVE)
        ssf = ss.rearrange("p h b -> p (h b)")
        half = st.tile([128, 32], F32, tag=f"half{g}", name=f"half{g}")
        nc.vector.tensor_scalar(
            out=half, in0=ssf, scalar1=0.5, scalar2=0.5 * EPS,
            op0=OP.mult, op1=OP.add,
        )
        rs = st.tile([128, 32, 1], F32, tag=f"rs{g}", name=f"rs{g}")
        rsf = rs.rearrange("p a one -> p (a one)")
        yi = rsf.bitcast(I32)
        nc.vector.tensor_scalar(
            out=yi, in0=ssf.bitcast(I32), scalar1=1, scalar2=None,
            op0=OP.logical_shift_right,
        )
        nc.vector.tensor_scalar(
            out=yi, in0=yi, scalar1=MAGIC, scalar2=-1,
            op0=OP.subtract, op1=OP.mult,
        )
        t1 = st.tile([128, 32], F32, tag=f"t1{g}", name=f"t1{g}")
        nc.vector.tensor_mul(t1, rsf, rsf)
        nc.vector.tensor_mul(t1, t1, half)
        nc.vector.tensor_scalar(
            out=t1, in0=t1, scalar1=1.5, scalar2=-sqrt_c2,
            op0=OP.subtract, op1=OP.mult,
        )
        nc.vector.tensor_mul(rsf, rsf, t1)

        # scale+cast to fp8 (one broadcast-multiply per head), transpose per
        # block (dispatch split across the SP and ACT queues for group 0),
        # then GpSimd-deinterleave into the DoubleRow block layout.
        q4 = qpool.tile([128, NB, 128], BF16, tag=f"q4_{g}", name=f"q4_{g}")
        qTi = qpool.tile([128, NB, 128], BF16, tag=f"qTi_{g}", name=f"qTi_{g}")
        qT = qpool.tile([128, N], BF16, tag=f"qT4_{g}", name=f"qT4_{g}")
        q4f = q4.bitcast(FP8)  # [128, NB, 256]
        qTif = qTi.bitcast(FP8)
        qTf = qT.bitcast(FP8).rearrange("p (two t) -> p two t", two=2)
        for hi in range(4):
            eng.tensor_mul(
                q4f[:, :, hi * 64 : (hi + 1) * 64],
                x4[:, hi],
                rs[:, hi * NB : (hi + 1) * NB, :].broadcast_to([128, NB, D]),
            )
        for b in range(NB):
            dq = nc.scalar if (g == 0 and b % 2 == 1) else nc.sync
            dq.dma_start(out=qTi[:, b, :], in_=q4[:, b, :], transpose=True)
            if b % 4 == 3:
                b0 = b - 3
                nc.gpsimd.tensor_copy(
                    qTf[:, :, b0 * 128 : (b0 + 4) * 128],
                    qTif[:, b0 : b0 + 4, :].rearrange(
                        "p b (t two) -> p two (b t)", two=2
                    ),
                )
        qT4[g] = qT

    def q_lhsT(g, hi, i):
        f = qT4[g].bitcast(FP8).rearrange("p (two t) -> p two t", two=2)
        return f[hi * 32 : (hi + 1) * 32, :, i * 128 : (i + 1) * 128]

    def q_rhs(g, hi, jc):
        f = qT4[g].bitcast(FP8).rearrange("p (two t) -> p two t", two=2)
        return f[hi * 32 : (hi + 1) * 32, :, jc * 512 : (jc + 1) * 512]

    wt_sb = singles.tile([D + 1, D], BF16)

    qprep(0)
    nc.sync.dma_start(out=wt_sb, in_=wt[:, :])

    def load_head(h):
        ya_t = io.tile([128, NB, D + 1], FP8, tag="ya", name=f"ya{h}")
        nc.sync.dma_start(
            out=ya_t, in_=ya[h].rearrange("(b p) d -> p b d", p=128)
        )
        yb_t = io.tile([128, NB, D], F32, tag="yb", name=f"yb{h}")
        nc.sync.dma_start(
            out=yb_t, in_=yb[h].rearrange("(b p) d -> p b d", p=128)
        )
        return ya_t, yb_t

    pend = load_head(0)

    for h in range(H):
        g, hi = h // 4, h % 4
        ya_t, yb_t = pend
        E = epool.tile([128, NW], FP8, tag="E", name=f"E{h}")
        Ei = E.bitcast(U8)
        po = ps_o.tile([128, 1024], F32, tag="o", name=f"o{h}")
        OT = wpool.tile([D + 1, N], BF16, tag="OT", name=f"OT{h}")
        okptr = [0]

        def emit_o(limit):
            while okptr[0] < 16:
                k = okptr[0]
                jc, i = k // NB, k % NB
                if jc * 4096 + (i + 1) * 512 > limit:
                    return
                nc.tensor.matmul(
                    po[0 : D + 1, jc * 512 : (jc + 1) * 512],
                    lhsT=ya_t[:, i, :],
                    rhs=E[:, jc * 4096 + i * 512 : jc * 4096 + (i + 1) * 512],
                    start=(i == 0), stop=(i == NB - 1), tile_position=(0, 0),
                )
                okptr[0] += 1

        def evac(jc):
            src = po[0 : D + 1, jc * 512 : (jc + 1) * 512]
            dst = OT[:, jc * 512 : (jc + 1) * 512]
            if eng_t["act"] + ACT_EVAC <= eng_t["dve"] + DVE_EVAC:
                nc.scalar.copy(out=dst, in_=src)
                eng_t["act"] += ACT_EVAC
            else:
                nc.vector.tensor_copy(dst, src)
                eng_t["dve"] += DVE_EVAC

        def emit_proj(jc):
            for bb in range(4):
                b = jc * 4 + bb
                nc.tensor.matmul(
                    po[:, jc * 512 + bb * 64 : jc * 512 + (bb + 1) * 64],
                    lhsT=OT[:, b * 128 : (b + 1) * 128],
                    rhs=wt_sb,
                    start=True, stop=True, tile_position=(0, 0),
                )

        for c in range(NB):
            jc, ip = c // 4, (c % 4) * 2
            ps = ps_s.tile([128, CHUNK], F32, tag="psS", name="psS")
            for k in range(2):
                nc.tensor.matmul(
                    ps[:, k * 512 : (k + 1) * 512],
                    lhsT=q_lhsT(g, hi, ip + k),
                    rhs=q_rhs(g, hi, jc),
                    start=True, stop=True,
                    perf_mode=PM.DoubleRow,
                    tile_position=(hi * 32, 0),
                )
            if eng_t["dve"] + DVE_CHUNK < eng_t["act"] + ACT_CHUNK:
                nc.vector.tensor_scalar(
                    out=Ei[:, c * CHUNK : (c + 1) * CHUNK],
                    in0=ps, scalar1=A_SCH, scalar2=B_SCH,
                    op0=OP.mult, op1=OP.add,
                )
                eng_t["dve"] += DVE_CHUNK
            else:
                nc.scalar.activation(
                    out=E[:, c * CHUNK : (c + 1) * CHUNK], in_=ps, func=ACT.Exp
                )
                eng_t["act"] += ACT_CHUNK
            emit_o(c * CHUNK)
            if c == 0:
                if h + 1 < H:
                    pend = load_head(h + 1)
                if h == 2:
                    qprep(1)
            elif c == 5:
                evac(0)
            elif c == 6:
                emit_proj(0)

        # ---- head tail: O flush, denominators, second half, epilogue ----
        emit_o(NW)
        evac(1)
        nc.sync.dma_start(out=rscr[h], in_=OT[D : D + 1, :])
        rT = st.tile([128, NB], BF16, tag="rT", name=f"rT{h}")
        nc.sync.dma_start(out=rT, in_=rscr[h].rearrange("(b p) -> p b", p=128))
        emit_proj(1)

        rinv = st.tile([128, NB, 1], F32, tag="rinv", name=f"rinv{h}")
        nc.vector.reciprocal(rinv.rearrange("p a one -> p (a one)"), rT)
        eng_t["dve"] += DVE_RECIP

        pj = (
            po.rearrange("p (jc x) -> p jc x", jc=2)[:, :, 0:256]
            .rearrange("p jc (bb d) -> p jc bb d", bb=4)
        )
        rb = (
            rinv.rearrange("p (jc bb) one -> p jc bb one", jc=2)
            .broadcast_to([128, 2, 4, D])
        )
        o2 = wpool.tile([128, NB, D], F32, tag="o2", name=f"o2{h}")
        nc.vector.tensor_mul(
            o2.rearrange("p (jc bb) d -> p jc bb d", jc=2), pj, rb
        )
        eng_t["dve"] += DVE_O2
        fin = wpool.tile([128, NB, D], F32, tag="fin", name=f"fin{h}")
        nc.vector.tensor_add(fin, o2, yb_t)
        eng_t["dve"] += 0.66
        nc.sync.dma_start(
            out=out[h].rearrange("(b p) d -> p b d", p=128), in_=fin
        )


def build_program(sqrt_c2: float) -> bass.Bass:
    from contextlib import ExitStack

    nc = bacc.Bacc(get_trn_type() or "TRN2", target_bir_lowering=False)
    with tile.TileContext(nc) as tc:
        with ExitStack() as ctx:
            _emit(ctx, tc, sqrt_c2)
    nc.compile()
    return nc


def make_inputs(x, y, proj_w, proj_b, attn_gamma, sum_gamma0, sum_gamma1):
    x = np.asarray(x, dtype=np.float32)
    y = np.asarray(y, dtype=np.float32)
    proj_w = np.asarray(proj_w, dtype=np.float32)
    proj_b = np.asarray(proj_b, dtype=np.float32)
    g0 = math.exp(float(np.asarray(sum_gamma0)))
    g1 = math.exp(float(np.asarray(sum_gamma1)))
    w0 = g0 / (g0 + g1)
    w1 = g1 / (g0 + g1)
    c2 = 1.0 / (SCALE * float(np.asarray(attn_gamma)))

    xq = x.astype(ml_dtypes.bfloat16)
    yac = np.concatenate(
        [y, np.ones(y.shape[:-1] + (1,), np.float32)], axis=-1
    ).astype(ml_dtypes.float8_e4m3fn)
    ybv = (w0 * y).astype(np.float32)
    wtv = np.concatenate([proj_w.T * w1, w1 * proj_b[None, :]], axis=0).astype(
        ml_dtypes.bfloat16
    )
    in_maps = [
        {"xq": xq[c], "ya": yac[c], "yb": ybv[c], "wt": wtv}
        for c in range(NCORES)
    ]
    return in_maps, math.sqrt(c2)


def kernel(x, y, proj_w, proj_b, attn_gamma, sum_gamma0, sum_gamma1):
    global LAST_RESULTS
    in_maps, sqrt_c2 = make_inputs(
        x, y, proj_w, proj_b, attn_gamma, sum_gamma0, sum_gamma1
    )
    nc = build_program(sqrt_c2)
    res = run_bass_kernel_spmd(nc, in_maps, list(range(NCORES)))
    LAST_RESULTS = res
    return np.stack([res.results[c]["out"] for c in range(NCORES)], axis=0)
